# revision 1
# baseline (speedup 1.0000x reference)
"""Multi-head causal attention (B=2, S=2048, D=1024, H=16) on 8 NeuronCores.

Sharding: core c = (batch b=c//4, head-group g=c%4 of 4 heads).
Each core projects Q/K (transposed layout) and V for its 4 heads from the
host-transposed input xT, runs causal attention over all 2048 queries in the
transposed-score layout ST[k, q] (all matmul operands in natural layout, no
on-device transposes), with the softmax denominator fused into the A@V matmul
via a ones-column appended to V. An 8-core AllToAll (split in two, one per
head pair, so the first overlaps attention of the second pair) swaps
head-shards for query-shards; every core then runs the output projection on a
fixed local 256-query slice of each batch. All matmuls use float32r
(full-rate fp32, ~1.5e-4 rel err).
"""

import numpy as np

import concourse.bass as bass
import concourse.mybir as mybir
import concourse.tile as tile
from concourse import bacc
from concourse.bass_utils import run_bass_kernel_spmd

B, S, D = 2, 2048, 1024
H = 16
DH = 64  # head dim
N_CORES = 8
GROUPS = 4  # cores per batch = head groups
H_LOC = H // GROUPS  # 4 heads per core
EH = H_LOC * DH  # 256 local qkv width
QCH = 512  # query chunk
NCH = S // QCH  # 4
KB = 128  # key block
NKB = S // KB  # 16
NDB = D // 128  # 8 contraction blocks
QL = 256  # local output query rows per batch
VW = DH + 1  # 65: V columns + fused ones column
SCALE = 1.0 / 8.0  # 1/sqrt(DH)

PACK_SCORES = False  # 2-head concurrent QK^T via PE row tiling
DEBUG_TAPS = False  # extra outputs: oft_own (pre-A2A) and oft_all (post-A2A)

F32 = mybir.dt.float32
F32R = mybir.dt.float32r
BF16 = mybir.dt.bfloat16
MM_DT = F32R  # dtype for all matmul operands
EXP = mybir.ActivationFunctionType.Exp
MULT = mybir.AluOpType.mult
ADD = mybir.AluOpType.add


def _emit(nc, tc, xT, wq_d, wk_d, wv_d, wo_d, bb_d, y_d):
    from contextlib import ExitStack

    ctx = ExitStack()
    with ctx:
        persist = ctx.enter_context(tc.tile_pool(name="persist", bufs=1))
        psum = ctx.enter_context(tc.tile_pool(name="psum", bufs=1, space="PSUM"))
        dram = ctx.enter_context(tc.tile_pool(name="dram", bufs=1, space="DRAM"))

        # --- constants: causal 0/1 masks ---
        # maskA: diag sub-blocks m=0,1 ; maskB: m=2,3. mask[ki, m*512+qi]=(qi>=ki+128m)
        mdt = F32 if MM_DT == F32R else MM_DT
        maskA = persist.tile([128, 2 * QCH], mdt)
        maskB = persist.tile([128, 2 * QCH], mdt)
        for mt, m0 in ((maskA, 0), (maskB, 2)):
            nc.gpsimd.memset(mt[:], 1.0)
            for sub in range(2):
                m = m0 + sub
                nc.gpsimd.affine_select(
                    out=mt[:, sub * QCH : (sub + 1) * QCH],
                    in_=mt[:, sub * QCH : (sub + 1) * QCH],
                    compare_op=mybir.AluOpType.is_ge,
                    fill=0.0,
                    base=-128 * m,
                    channel_multiplier=-1,
                    pattern=[[1, QCH]],
                )
        ones_f = persist.tile([128, 1], F32)
        nc.gpsimd.memset(ones_f[:], 1.0)
        bb_sb = persist.tile([128, D], F32)
        nc.sync.dma_start(bb_sb[:], bb_d[:])

        # attention outputs for own heads, transposed: 2 tiles x [128 (2 heads), S]
        oft_own = [persist.tile([128, S], MM_DT, name=f"oftown{p}") for p in range(2)]

        qkvp = ctx.enter_context(tc.tile_pool(name="qkvp", bufs=1))
        qt = [qkvp.tile([128, S], MM_DT, name=f"qt{p}") for p in range(2)]
        kt = [qkvp.tile([128, S], MM_DT, name=f"kt{p}") for p in range(2)]
        vg = [qkvp.tile([128, NKB * VW], MM_DT, name=f"vg{h}") for h in range(H_LOC)]
        for h in range(H_LOC):
            nc.vector.tensor_copy(
                vg[h].rearrange("p (n w) -> p n w", w=VW)[:, :, DH : DH + 1],
                ones_f[:].unsqueeze(2).broadcast_to([128, NKB, 1]),
            )

        exps = ctx.enter_context(tc.tile_pool(name="exps", bufs=1))

        def normalize(h, pot, j):
            p, r = h // 2, DH * (h % 2)
            den = exps.tile([1, QCH], F32, tag="den", bufs=2, name=f"den{h}_{j}")
            nc.vector.tensor_copy(den[:], pot[DH : DH + 1, :])
            rec = exps.tile([1, QCH], F32, tag="rec", bufs=2, name=f"rec{h}_{j}")
            nc.vector.reciprocal_approx_fast(rec[:], den[:])
            pb_sb = exps.tile([DH, QCH], F32, tag="pbsb", bufs=2, name=f"pb{h}_{j}")
            nc.gpsimd.partition_broadcast(pb_sb[:], rec[0:1, :])
            nc.vector.tensor_tensor(
                oft_own[p][r : r + DH, j * QCH : (j + 1) * QCH],
                pot[0:DH, :],
                pb_sb[:],
                op=MULT,
            )

        def emit_attention_pair(p, mid=None):
            h0, h1 = 2 * p, 2 * p + 1
            for j in range(NCH):
                nkb_j = 4 * (j + 1)
                pot = [
                    psum.tile([VW, QCH], F32, tag="pot", bufs=2, name=f"pot{h}_{j}")
                    for h in (h0, h1)
                ]
                for g in range(2 * (j + 1)):
                    pss = [
                        psum.tile(
                            [128, 2 * QCH], F32, tag="ps", bufs=3, name=f"ps{h}{j}{g}"
                        )
                        for h in (h0, h1)
                    ]
                    for sub in range(2):
                        kb = 2 * g + sub
                        m = kb - 4 * j
                        off = 128 * m if (g >= 2 * j and m > 0) else 0
                        for hi in range(2):
                            nc.tensor.matmul(
                                pss[hi][:, sub * QCH + off : (sub + 1) * QCH],
                                kt[p][
                                    hi * DH : (hi + 1) * DH, kb * KB : (kb + 1) * KB
                                ],
                                qt[p][
                                    hi * DH : (hi + 1) * DH,
                                    j * QCH + off : (j + 1) * QCH,
                                ],
                                start=True,
                                stop=True,
                                tile_position=(hi * DH, 0) if PACK_SCORES else None,
                            )
                    for hi, h in enumerate((h0, h1)):
                        e = exps.tile(
                            [128, 2 * QCH], MM_DT, tag="exp", bufs=3, name=f"e{h}{j}{g}"
                        )
                        # diagonal groups: leading 128*m0 columns of sub0 are
                        # fully masked; skip them in the exp (the mask multiply
                        # zeroes them from whatever stale data remains)
                        eoff = 128 * (2 * g - 4 * j) if g >= 2 * j else 0
                        nc.scalar.activation(
                            e[:, eoff:], pss[hi][:, eoff:], EXP, scale=SCALE
                        )
                        if g >= 2 * j:
                            mt = maskA if g == 2 * j else maskB
                            nc.vector.tensor_tensor(
                                e[:], e[:].bitcast(F32) if MM_DT == F32R else e[:],
                                mt[:], op=MULT,
                            )
                        for sub in range(2):
                            kb = 2 * g + sub
                            m = kb - 4 * j
                            off = 128 * m if (g >= 2 * j and m > 0) else 0
                            nc.tensor.matmul(
                                pot[hi][:, off:QCH],
                                vg[h][:, kb * VW : (kb + 1) * VW],
                                e[:, sub * QCH + off : (sub + 1) * QCH],
                                start=(kb == 0),
                                stop=(kb == nkb_j - 1),
                            )
                normalize(h0, pot[0], j)
                normalize(h1, pot[1], j)
                if mid is not None and j in mid:
                    mid[j]()

        with tc.tile_pool(name="xtw", bufs=1) as xtw:
            # weights first (small, gate the first projections), then xT
            w_sb = {
                nm: xtw.tile([128, NDB * EH], MM_DT, name=f"w{nm}sb")
                for nm in ("q", "k", "v")
            }
            for nm, wd in (("q", wq_d), ("k", wk_d), ("v", wv_d)):
                nc.gpsimd.dma_start(
                    w_sb[nm][:].rearrange("p (db e) -> p db e", db=NDB),
                    wd.rearrange("(db p) e -> p db e", p=128),
                )
            xt_sb = [xtw.tile([128, S], MM_DT, name=f"xt{d}") for d in range(NDB)]
            dma_engs = [nc.sync, nc.scalar]
            for d in range(NDB):
                for hf in range(2):
                    stg = xtw.tile(
                        [128, S // 2], F32, tag="xstage", bufs=3, name=f"xs{d}_{hf}"
                    )
                    dma_engs[(2 * d + hf) % 2].dma_start(
                        stg[:],
                        xT[d * 128 : (d + 1) * 128, hf * (S // 2) : (hf + 1) * (S // 2)],
                    )
                    nc.vector.tensor_copy(
                        xt_sb[d][:, hf * (S // 2) : (hf + 1) * (S // 2)], stg[:]
                    )

            def emit_proj_qk(p):
                for dst, wsb in ((qt[p], w_sb["q"]), (kt[p], w_sb["k"])):
                    for j in range(NCH):
                        ps = psum.tile(
                            [128, QCH], F32, tag="ps", bufs=3, name=f"pp{p}_{j}"
                        )
                        for d in range(NDB):
                            nc.tensor.matmul(
                                ps[:],
                                wsb[:, d * EH + 128 * p : d * EH + 128 * p + 128],
                                xt_sb[d][:, j * QCH : (j + 1) * QCH],
                                start=(d == 0),
                                stop=(d == NDB - 1),
                            )
                        nc.vector.tensor_copy(dst[:, j * QCH : (j + 1) * QCH], ps[:])

            def emit_proj_v():
                for sb_i in range(NKB):
                    ps = psum.tile([128, EH], F32, tag="ps", bufs=3, name=f"pv{sb_i}")
                    for d in range(NDB):
                        nc.tensor.matmul(
                            ps[:],
                            xt_sb[d][:, sb_i * KB : (sb_i + 1) * KB],
                            w_sb["v"][:, d * EH : (d + 1) * EH],
                            start=(d == 0),
                            stop=(d == NDB - 1),
                        )
                    for h in range(H_LOC):
                        nc.vector.tensor_copy(
                            vg[h][:, sb_i * VW : sb_i * VW + DH],
                            ps[:, h * DH : (h + 1) * DH],
                        )

            emit_proj_qk(0)
            emit_proj_v()
            emit_attention_pair(0)
            emit_proj_qk(1)

        # --- AllToAll #1 (head pair 0) + output-projection setup ---
        # shard s of pair-p buffer = this core's 2 heads' columns q in
        # [256s, 256s+256). Received shard from rank r lands at rows
        # [128r, 128r+128) = global f-rows [256r + 128p, 256r + 128p + 128)
        # = f-block 2r+p. Rows [0,1024) of the combined view = batch 0,
        # [1024, 2048) = batch 1 -- uniform on every core.
        oftp = ctx.enter_context(tc.tile_pool(name="oftp", bufs=1))
        wo_sb = oftp.tile([128, NDB * D], MM_DT)
        oft_all = [oftp.tile([128, 2 * QL], MM_DT, name=f"oft{f}") for f in range(NDB)]
        nc.gpsimd.dma_start(
            wo_sb[:].rearrange("p (db e) -> p db e", db=NDB),
            wo_d.rearrange("(db p) e -> p db e", p=128),
        )

        a2a_bufs = {}

        def emit_a2a_pre(p):
            cdt = F32 if MM_DT == F32R else MM_DT
            cin = dram.tile([N_CORES * 128, QL], cdt, name=f"cin{p}")
            cout = dram.tile([N_CORES * 128, QL], cdt, name=f"cout{p}")
            a2a_bufs[p] = (cin, cout)
            cin_eng = nc.sync if p == 0 else nc.scalar
            for s in range(N_CORES):
                cin_eng.dma_start(
                    cin[s * 128 : (s + 1) * 128, :],
                    oft_own[p][:, s * QL : (s + 1) * QL].bitcast(F32)
                    if MM_DT == F32R
                    else oft_own[p][:, s * QL : (s + 1) * QL],
                )

        def emit_a2a_trigger(p):
            cin, cout = a2a_bufs[p]
            nc.gpsimd.collective_compute(
                "AllToAll",
                mybir.AluOpType.bypass,
                replica_groups=[list(range(N_CORES))],
                ins=[cin[:]],
                outs=[cout[:]],
            )

        def emit_a2a_post(p):
            cin, cout = a2a_bufs[p]
            for r in range(GROUPS):
                for bi in range(2):
                    src_rank = bi * GROUPS + r
                    if MM_DT == F32R:
                        rstg = oftp.tile(
                            [128, QL], F32, tag="rstg", bufs=3, name=f"rs{p}{r}{bi}"
                        )
                        eng = nc.sync if p == 0 else nc.scalar
                        eng.dma_start(
                            rstg[:], cout[src_rank * 128 : (src_rank + 1) * 128, :]
                        )
                        nc.vector.tensor_copy(
                            oft_all[2 * r + p][:, bi * QL : (bi + 1) * QL], rstg[:]
                        )
                    else:
                        eng = nc.sync if p == 0 else nc.scalar
                        eng.dma_start(
                            oft_all[2 * r + p][:, bi * QL : (bi + 1) * QL],
                            cout[src_rank * 128 : (src_rank + 1) * 128, :],
                        )

        emit_a2a_pre(0)
        emit_attention_pair(
            1,
            mid={
                1: lambda: (emit_a2a_trigger(0), emit_a2a_post(0)),
            },
        )
        emit_a2a_pre(1)
        emit_a2a_trigger(1)
        emit_a2a_post(1)

        if DEBUG_TAPS and MM_DT == F32R:
            dbg_own = nc.dram_tensor("dbg_own", [256, S], F32, kind="ExternalOutput")
            for p in range(2):
                nc.sync.dma_start(
                    dbg_own.ap()[p * 128 : (p + 1) * 128, :],
                    oft_own[p][:].bitcast(F32),
                )
            dbg_all = nc.dram_tensor(
                "dbg_all", [D, 2 * QL], F32, kind="ExternalOutput"
            )
            for f in range(NDB):
                nc.sync.dma_start(
                    dbg_all.ap()[f * 128 : (f + 1) * 128, :],
                    oft_all[f][:].bitcast(F32),
                )

        # --- output projection on local 256-query slice of each batch ---
        for bi in range(2):
            for qb in range(QL // 128):
                ysb = oftp.tile([128, D], F32, tag="ysb", bufs=2, name=f"y{bi}_{qb}")
                for ech in range(2):
                    py = psum.tile(
                        [128, 512], F32, tag="ps", bufs=3, name=f"py{bi}{qb}{ech}"
                    )
                    forder = [0, 2, 4, 6, 1, 3, 5, 7]
                    for fi, f in enumerate(forder):
                        nc.tensor.matmul(
                            py[:],
                            oft_all[f][
                                :, bi * QL + qb * 128 : bi * QL + (qb + 1) * 128
                            ],
                            wo_sb[:, f * D + ech * 512 : f * D + ech * 512 + 512],
                            start=(fi == 0),
                            stop=(fi == NDB - 1),
                        )
                    nc.vector.tensor_tensor(
                        ysb[:, ech * 512 : (ech + 1) * 512],
                        py[:],
                        bb_sb[:, ech * 512 : (ech + 1) * 512],
                        op=ADD,
                    )
                nc.sync.dma_start(
                    y_d[bi * QL + qb * 128 : bi * QL + (qb + 1) * 128, :], ysb[:]
                )


def build_program():
    nc = bacc.Bacc(
        "TRN2", target_bir_lowering=False, debug=False, num_devices=N_CORES
    )
    xT = nc.dram_tensor("xT", [D, S], F32, kind="ExternalInput")
    wq = nc.dram_tensor("wq", [D, EH], F32, kind="ExternalInput")
    wk = nc.dram_tensor("wk", [D, EH], F32, kind="ExternalInput")
    wv = nc.dram_tensor("wv", [D, EH], F32, kind="ExternalInput")
    wo = nc.dram_tensor("wo", [D, D], F32, kind="ExternalInput")
    bb = nc.dram_tensor("bb", [128, D], F32, kind="ExternalInput")
    y = nc.dram_tensor("y", [2 * QL, D], F32, kind="ExternalOutput")
    with tile.TileContext(nc) as tc:
        _emit(nc, tc, xT.ap(), wq.ap(), wk.ap(), wv.ap(), wo.ap(), bb.ap(), y.ap())
    nc.compile()
    return nc


_cached_nc = None


def _get_nc():
    global _cached_nc
    if _cached_nc is None:
        _cached_nc = build_program()
    return _cached_nc


def make_in_maps(x, w_qkv, w_out, b_out):
    x = np.ascontiguousarray(np.asarray(x, np.float32))
    w_qkv = np.asarray(w_qkv, np.float32)
    w_out = np.ascontiguousarray(np.asarray(w_out, np.float32))
    b_out = np.asarray(b_out, np.float32)
    bb = np.ascontiguousarray(np.broadcast_to(b_out, (128, D)))
    in_maps = []
    for c in range(N_CORES):
        b, g = c // GROUPS, c % GROUPS
        in_maps.append(
            {
                "xT": np.ascontiguousarray(x[b].T),
                "wq": np.ascontiguousarray(w_qkv[:, g * EH : (g + 1) * EH]),
                "wk": np.ascontiguousarray(w_qkv[:, D + g * EH : D + (g + 1) * EH]),
                "wv": np.ascontiguousarray(
                    w_qkv[:, 2 * D + g * EH : 2 * D + (g + 1) * EH]
                ),
                "wo": w_out,
                "bb": bb,
            }
        )
    return in_maps


def assemble(results):
    # core c's y is [512, D]: rows [0,256) = batch 0 q-slice [256c, 256c+256),
    # rows [256,512) = batch 1 same slice.
    y = np.empty((B, S, D), np.float32)
    for c in range(N_CORES):
        yc = results[c]["y"]
        y[0, 256 * c : 256 * (c + 1), :] = yc[:256]
        y[1, 256 * c : 256 * (c + 1), :] = yc[256:]
    return y


def kernel(x, w_qkv, w_out, b_out, _trace=False, **run_kwargs):
    nc = _get_nc()
    in_maps = make_in_maps(x, w_qkv, w_out, b_out)
    res = run_bass_kernel_spmd(
        nc, in_maps, core_ids=list(range(N_CORES)), trace=_trace, **run_kwargs
    )
    out = assemble(res.results)
    if _trace:
        return out, res
    return out



# revision 3
# speedup vs baseline: 1.2906x; 1.2906x over previous
"""Multi-head causal attention (B=2, S=2048, D=1024, H=16) on 8 NeuronCores.

Sharding: core c = (batch b=c//4, head-group g=c%4 of 4 heads).
All matmul operands in bf16 (host pre-casts x/w_qkv/w_out), fp32 PSUM
accumulation. Per core: project Q/K (transposed layout) and V for its 4
heads, run causal attention over all 2048 queries in transposed-score
layout ST[k, q], softmax denominator fused into the A@V matmul via a
ones-column in V. Two AllToAlls (one per head pair) swap head-shards for
query-shards; each core then runs the output projection on a fixed local
256-query slice of each batch.

Schedule: pair-1 Q/K projection is interleaved into pair-0 attention via
mid-chunk callbacks; A2A#1 triggers right after pair-0 attention and its
results land while pair-1 attention runs; the output projection
accumulates even f-blocks (from A2A#1) first so only odd blocks wait on
A2A#2. The scalar queue carries only exp; all collective staging uses
direct DMA on the sync queue so no compute FIFO blocks on a collective.
"""

import numpy as np
import ml_dtypes

import concourse.bass as bass
import concourse.mybir as mybir
import concourse.tile as tile
from concourse import bacc
from concourse.bass_utils import run_bass_kernel_spmd

B, S, D = 2, 2048, 1024
H = 16
DH = 64  # head dim
N_CORES = 8
GROUPS = 4  # cores per batch = head groups
H_LOC = H // GROUPS  # 4 heads per core
EH = H_LOC * DH  # 256 local qkv width
QCH = 512  # query chunk
NCH = S // QCH  # 4
KB = 128  # key block
NKB = S // KB  # 16
NDB = D // 128  # 8 contraction blocks
QL = 256  # local output query rows per batch
VW = DH + 1  # 65: V columns + fused ones column
SCALE = 1.0 / 8.0  # 1/sqrt(DH)

F32 = mybir.dt.float32
BF16 = mybir.dt.bfloat16
MM_DT = BF16
EXP = mybir.ActivationFunctionType.Exp
MULT = mybir.AluOpType.mult
ADD = mybir.AluOpType.add


def _emit(nc, tc, xT, wq_d, wk_d, wv_d, wo_d, bb_d, y_d):
    from contextlib import ExitStack

    ctx = ExitStack()
    with ctx:
        persist = ctx.enter_context(tc.tile_pool(name="persist", bufs=1))
        psum = ctx.enter_context(tc.tile_pool(name="psum", bufs=1, space="PSUM"))
        dram = ctx.enter_context(tc.tile_pool(name="dram", bufs=1, space="DRAM"))

        # --- weights first (gpsimd queue), x chunks (sync queue) ---
        w_sb = {
            nm: persist.tile([128, NDB * EH], MM_DT, name=f"w{nm}sb")
            for nm in ("q", "k", "v")
        }
        for nm, wd in (("q", wq_d), ("k", wk_d), ("v", wv_d)):
            nc.gpsimd.dma_start(
                w_sb[nm][:].rearrange("p (db e) -> p db e", db=NDB),
                wd.rearrange("(db p) e -> p db e", p=128),
            )
        xt = persist.tile([128, NDB * S], MM_DT, name="xt")
        xt_v = xt[:].rearrange("p (db s) -> p db s", db=NDB)
        xT_v = xT.rearrange("(db p) s -> p db s", p=128)
        for j in range(NCH):
            nc.sync.dma_start(
                xt_v[:, :, j * QCH : (j + 1) * QCH],
                xT_v[:, :, j * QCH : (j + 1) * QCH],
            )

        # --- constants: causal 0/1 masks ---
        # maskA: diag sub-blocks m=0,1 ; maskB: m=2,3. mask[ki, m*512+qi]=(qi>=ki+128m)
        maskA = persist.tile([128, 2 * QCH], MM_DT)
        maskB = persist.tile([128, 2 * QCH], MM_DT)
        for mt, m0 in ((maskA, 0), (maskB, 2)):
            nc.gpsimd.memset(mt[:], 1.0)
            for sub in range(2):
                m = m0 + sub
                nc.gpsimd.affine_select(
                    out=mt[:, sub * QCH : (sub + 1) * QCH],
                    in_=mt[:, sub * QCH : (sub + 1) * QCH],
                    compare_op=mybir.AluOpType.is_ge,
                    fill=0.0,
                    base=-128 * m,
                    channel_multiplier=-1,
                    pattern=[[1, QCH]],
                )
        ones_b = persist.tile([128, 1], MM_DT)
        nc.gpsimd.memset(ones_b[:], 1.0)

        # V with fused ones column: slice (kb, h) at (kb*H_LOC + h) * VW
        vgall = persist.tile([128, NKB * H_LOC * VW], MM_DT, name="vgall")
        nc.vector.tensor_copy(
            vgall[:].rearrange("p (n w) -> p n w", w=VW)[:, :, DH : DH + 1],
            ones_b[:].unsqueeze(2).broadcast_to([128, NKB * H_LOC, 1]),
        )

        def vg(h, kb):
            i = (kb * H_LOC + h) * VW
            return vgall[:, i : i + VW]

        # output-projection weights (needed from ~mid-kernel) + bias
        wo_sb = persist.tile([128, NDB * D], MM_DT)
        nc.gpsimd.dma_start(
            wo_sb[:].rearrange("p (db e) -> p db e", db=NDB),
            wo_d.rearrange("(db p) e -> p db e", p=128),
        )
        bb_sb = persist.tile([128, D], F32)
        nc.sync.dma_start(bb_sb[:], bb_d[:])

        # attention outputs for own heads, transposed: 2 tiles x [128 (2 heads), S]
        oft_own = [persist.tile([128, S], MM_DT, name=f"oftown{p}") for p in range(2)]
        qt = [persist.tile([128, S], MM_DT, name=f"qt{p}") for p in range(2)]
        kt = [persist.tile([128, S], MM_DT, name=f"kt{p}") for p in range(2)]

        exps = ctx.enter_context(tc.tile_pool(name="exps", bufs=1))

        def normalize(h, pot, j):
            p, r = h // 2, DH * (h % 2)
            den = exps.tile([1, QCH], F32, tag="den", bufs=2, name=f"den{h}_{j}")
            nc.vector.tensor_copy(den[:], pot[DH : DH + 1, :])
            rec = exps.tile([1, QCH], F32, tag="rec", bufs=2, name=f"rec{h}_{j}")
            nc.vector.reciprocal_approx_fast(rec[:], den[:])
            pb_sb = exps.tile([DH, QCH], F32, tag="pbsb", bufs=2, name=f"pb{h}_{j}")
            nc.gpsimd.partition_broadcast(pb_sb[:], rec[0:1, :])
            nc.vector.tensor_tensor(
                oft_own[p][r : r + DH, j * QCH : (j + 1) * QCH],
                pot[0:DH, :],
                pb_sb[:],
                op=MULT,
            )

        def emit_attention_pair(p, mid=None):
            h0, h1 = 2 * p, 2 * p + 1
            for j in range(NCH):
                nkb_j = 4 * (j + 1)
                pot = [
                    psum.tile([VW, QCH], F32, tag="pot", bufs=2, name=f"pot{h}_{j}")
                    for h in (h0, h1)
                ]
                for g in range(2 * (j + 1)):
                    pss = [
                        psum.tile(
                            [128, 2 * QCH], F32, tag="ps", bufs=3, name=f"ps{h}{j}{g}"
                        )
                        for h in (h0, h1)
                    ]
                    for sub in range(2):
                        kb = 2 * g + sub
                        m = kb - 4 * j
                        off = 128 * m if (g >= 2 * j and m > 0) else 0
                        for hi in range(2):
                            nc.tensor.matmul(
                                pss[hi][:, sub * QCH + off : (sub + 1) * QCH],
                                kt[p][
                                    hi * DH : (hi + 1) * DH, kb * KB : (kb + 1) * KB
                                ],
                                qt[p][
                                    hi * DH : (hi + 1) * DH,
                                    j * QCH + off : (j + 1) * QCH,
                                ],
                                start=True,
                                stop=True,
                            )
                    for hi, h in enumerate((h0, h1)):
                        e = exps.tile(
                            [128, 2 * QCH], MM_DT, tag="exp", bufs=3, name=f"e{h}{j}{g}"
                        )
                        # diagonal groups: leading 128*m0 columns of sub0 are
                        # fully masked; skip them in the exp (the mask multiply
                        # zeroes them from whatever stale data remains)
                        eoff = 128 * (2 * g - 4 * j) if g >= 2 * j else 0
                        nc.scalar.activation(
                            e[:, eoff:], pss[hi][:, eoff:], EXP, scale=SCALE
                        )
                        if g >= 2 * j:
                            mt = maskA if g == 2 * j else maskB
                            nc.vector.tensor_tensor(e[:], e[:], mt[:], op=MULT)
                        for sub in range(2):
                            kb = 2 * g + sub
                            m = kb - 4 * j
                            off = 128 * m if (g >= 2 * j and m > 0) else 0
                            nc.tensor.matmul(
                                pot[hi][:, off:QCH],
                                vg(h, kb),
                                e[:, sub * QCH + off : (sub + 1) * QCH],
                                start=(kb == 0),
                                stop=(kb == nkb_j - 1),
                            )
                normalize(h0, pot[0], j)
                normalize(h1, pot[1], j)
                if mid is not None and j in mid:
                    mid[j]()

        def emit_proj_qk_unit(p, dst_i, j):
            dst, wsb = ((qt[p], w_sb["q"]), (kt[p], w_sb["k"]))[dst_i]
            ps = psum.tile([128, QCH], F32, tag="ps", bufs=3, name=f"pp{p}{dst_i}{j}")
            for d in range(NDB):
                nc.tensor.matmul(
                    ps[:],
                    wsb[:, d * EH + 128 * p : d * EH + 128 * p + 128],
                    xt[:, d * S + j * QCH : d * S + (j + 1) * QCH],
                    start=(d == 0),
                    stop=(d == NDB - 1),
                )
            nc.vector.tensor_copy(dst[:, j * QCH : (j + 1) * QCH], ps[:])

        def emit_proj_qk(p):
            for dst_i in range(2):
                for j in range(NCH):
                    emit_proj_qk_unit(p, dst_i, j)

        def emit_proj_v():
            for sb_i in range(NKB):
                ps = psum.tile([128, EH], F32, tag="ps", bufs=3, name=f"pv{sb_i}")
                for d in range(NDB):
                    nc.tensor.matmul(
                        ps[:],
                        xt[:, d * S + sb_i * KB : d * S + (sb_i + 1) * KB],
                        w_sb["v"][:, d * EH : (d + 1) * EH],
                        start=(d == 0),
                        stop=(d == NDB - 1),
                    )
                dst = vgall[:, sb_i * H_LOC * VW : (sb_i + 1) * H_LOC * VW]
                nc.vector.tensor_copy(
                    dst.rearrange("p (h w) -> p h w", w=VW)[:, :, 0:DH],
                    ps[:].rearrange("p (h d) -> p h d", d=DH),
                )

        # --- AllToAll: shard s of pair-p buffer = this core's 2 heads' columns
        # q in [256s, 256s+256). Received shard from rank r lands at rows
        # [128r, 128r+128) = global f-rows [256r + 128p, +128) = f-block 2r+p.
        oft_all = [
            persist.tile([128, 2 * QL], MM_DT, name=f"oft{f}") for f in range(NDB)
        ]
        a2a_bufs = {}

        def emit_a2a_pre(p):
            cin = dram.tile([N_CORES * 128, QL], MM_DT, name=f"cin{p}")
            cout = dram.tile([N_CORES * 128, QL], MM_DT, name=f"cout{p}")
            a2a_bufs[p] = (cin, cout)
            nc.sync.dma_start(
                cin.rearrange("(s p) q -> p s q", p=128),
                oft_own[p][:].rearrange("p (s q) -> p s q", q=QL),
            )

        def emit_a2a_trigger(p):
            cin, cout = a2a_bufs[p]
            nc.gpsimd.collective_compute(
                "AllToAll",
                mybir.AluOpType.bypass,
                replica_groups=[list(range(N_CORES))],
                ins=[cin[:]],
                outs=[cout[:]],
            )

        def emit_a2a_post(p):
            cin, cout = a2a_bufs[p]
            cout_v = cout.rearrange("(b r p) q -> p b r q", p=128, b=2)
            for r in range(GROUPS):
                nc.sync.dma_start(
                    oft_all[2 * r + p][:].rearrange("p (b q) -> p b q", q=QL),
                    cout_v[:, :, r, :],
                )

        # --- schedule ---
        emit_proj_qk(0)
        emit_proj_v()
        emit_attention_pair(
            0,
            mid={
                0: lambda: (emit_proj_qk_unit(1, 0, 0), emit_proj_qk_unit(1, 0, 1)),
                1: lambda: (emit_proj_qk_unit(1, 0, 2), emit_proj_qk_unit(1, 0, 3)),
                2: lambda: (emit_proj_qk_unit(1, 1, 0), emit_proj_qk_unit(1, 1, 1)),
                3: lambda: (emit_proj_qk_unit(1, 1, 2), emit_proj_qk_unit(1, 1, 3)),
            },
        )
        emit_a2a_pre(0)
        emit_a2a_trigger(0)
        emit_a2a_post(0)
        emit_attention_pair(1)
        emit_a2a_pre(1)
        emit_a2a_trigger(1)
        emit_a2a_post(1)

        # --- output projection on local 256-query slice of each batch ---
        # even f-blocks (from A2A#1) accumulate first; odd blocks wait on A2A#2
        for bi in range(2):
            for qb in range(QL // 128):
                ysb = persist.tile([128, D], F32, tag="ysb", bufs=2, name=f"y{bi}{qb}")
                for ech in range(2):
                    py = psum.tile(
                        [128, 512], F32, tag="ps", bufs=3, name=f"py{bi}{qb}{ech}"
                    )
                    forder = [0, 2, 4, 6, 1, 3, 5, 7]
                    for fi, f in enumerate(forder):
                        nc.tensor.matmul(
                            py[:],
                            oft_all[f][
                                :, bi * QL + qb * 128 : bi * QL + (qb + 1) * 128
                            ],
                            wo_sb[:, f * D + ech * 512 : f * D + ech * 512 + 512],
                            start=(fi == 0),
                            stop=(fi == NDB - 1),
                        )
                    nc.vector.tensor_tensor(
                        ysb[:, ech * 512 : (ech + 1) * 512],
                        py[:],
                        bb_sb[:, ech * 512 : (ech + 1) * 512],
                        op=ADD,
                    )
                nc.sync.dma_start(
                    y_d[bi * QL + qb * 128 : bi * QL + (qb + 1) * 128, :], ysb[:]
                )


def build_program():
    nc = bacc.Bacc(
        "TRN2", target_bir_lowering=False, debug=False, num_devices=N_CORES
    )
    xT = nc.dram_tensor("xT", [D, S], BF16, kind="ExternalInput")
    wq = nc.dram_tensor("wq", [D, EH], BF16, kind="ExternalInput")
    wk = nc.dram_tensor("wk", [D, EH], BF16, kind="ExternalInput")
    wv = nc.dram_tensor("wv", [D, EH], BF16, kind="ExternalInput")
    wo = nc.dram_tensor("wo", [D, D], BF16, kind="ExternalInput")
    bb = nc.dram_tensor("bb", [128, D], F32, kind="ExternalInput")
    y = nc.dram_tensor("y", [2 * QL, D], F32, kind="ExternalOutput")
    with tile.TileContext(nc) as tc:
        _emit(nc, tc, xT.ap(), wq.ap(), wk.ap(), wv.ap(), wo.ap(), bb.ap(), y.ap())
    nc.compile()
    return nc


_cached_nc = None


def _get_nc():
    global _cached_nc
    if _cached_nc is None:
        _cached_nc = build_program()
    return _cached_nc


def make_in_maps(x, w_qkv, w_out, b_out):
    bf = ml_dtypes.bfloat16
    x = np.asarray(x, np.float32)
    w_qkv = np.asarray(w_qkv, np.float32).astype(bf)
    w_out = np.ascontiguousarray(np.asarray(w_out, np.float32).astype(bf))
    b_out = np.asarray(b_out, np.float32)
    bb = np.ascontiguousarray(np.broadcast_to(b_out, (128, D)))
    xTb = [np.ascontiguousarray(x[b].T.astype(bf)) for b in range(B)]
    in_maps = []
    for c in range(N_CORES):
        b, g = c // GROUPS, c % GROUPS
        in_maps.append(
            {
                "xT": xTb[b],
                "wq": np.ascontiguousarray(w_qkv[:, g * EH : (g + 1) * EH]),
                "wk": np.ascontiguousarray(w_qkv[:, D + g * EH : D + (g + 1) * EH]),
                "wv": np.ascontiguousarray(
                    w_qkv[:, 2 * D + g * EH : 2 * D + (g + 1) * EH]
                ),
                "wo": w_out,
                "bb": bb,
            }
        )
    return in_maps


def assemble(results):
    # core c's y is [512, D]: rows [0,256) = batch 0 q-slice [256c, 256c+256),
    # rows [256,512) = batch 1 same slice.
    y = np.empty((B, S, D), np.float32)
    for c in range(N_CORES):
        yc = results[c]["y"]
        y[0, 256 * c : 256 * (c + 1), :] = yc[:256]
        y[1, 256 * c : 256 * (c + 1), :] = yc[256:]
    return y


def kernel(x, w_qkv, w_out, b_out, _trace=False, **run_kwargs):
    nc = _get_nc()
    in_maps = make_in_maps(x, w_qkv, w_out, b_out)
    res = run_bass_kernel_spmd(
        nc, in_maps, core_ids=list(range(N_CORES)), trace=_trace, **run_kwargs
    )
    out = assemble(res.results)
    if _trace:
        return out, res
    return out
